# revision 24
# baseline (speedup 1.0000x reference)
"""Trainium2 Bass kernel for the AGCA channel-gating module (gnn_message_passing).

Reference computation (per batch element b):
    m   = mean(x[b], over H,W)                  # (C,)
    y1  = w1 @ m                                # (HIDE,)
    s   = softmax(w2 * y1)                      # (HIDE,)
    y2  = y1 * s + A2.T @ y1                    # (HIDE,)
    y3  = relu(w3 * y2)                         # (HIDE,)
    g   = sigmoid(w4 @ y3)                      # (C,)
    out[b] = x[b] * g[:, None, None]

Memory-bound: 256 MB in + 256 MB out.  Strategy: pure data parallel over
batch (2 batch elements per NeuronCore).  Each 16 MB batch element is held
fully in SBUF so x is read exactly once.  Per core the pipeline is:

  stream 2 MB loads (sync HWDGE ring, ~428 GB/s)
    -> one free-dim partial-sum per tile, alternating DVE reduce_sum /
       ACT accumulate-copy so neither engine falls behind the 4.65 us/tile
       load pace
    -> tiny gate math (PE matmuls on the partial-sum columns, softmax via
       exp + ones-matmul partition sum + broadcast matmul, relu fused into
       a DVE tensor_scalar, one sigmoid over both gate columns)
    -> in-place per-channel scale (DVE/ACT alternating)
    -> stream stores (scalar HWDGE ring; second batch alternates rings).

Batch 1's loads/reduces are emitted interleaved with batch 0's muls/stores
so each engine's FIFO alternates between the two streams (engines execute
in emission order; a blocked stream would otherwise stall the other).  The
measured result is fabric-port saturation (~430 GB/s) for the whole kernel.

All weights/constants are packed into one DRAM tensor ("wpack") loaded by a
single DMA on the gpsimd (SWDGE) queue, and each compute engine "warms up"
on it once so real instructions carry at most one sync wait (walrus's
instruction encodings fit only one; Bacc legalizes the rest).  The Exp and
Sigmoid ACT tables are pre-warmed so no table load lands on the gate
chain's critical path.
"""

import numpy as np

import concourse.bass as bass
import concourse.mybir as mybir
import concourse.tile as tile
from concourse import bacc
from concourse.bass_utils import run_bass_kernel_spmd

B, C, H, W = 16, 256, 128, 128
HIDE = C // 2          # 128
NCORES = 8
BPC = B // NCORES      # batch elements per core = 2
HW = H * W             # 16384
P = 128                # SBUF partitions; C = 2 * P
LCHUNK = 4             # 2 MB chunks per channel half
F = HW // LCHUNK       # 4096
CHUNKS = [(j * F, F) for j in range(LCHUNK)]   # per-half (start, width)
NCH = len(CHUNKS)      # 4 per half, 8 per batch element
XBUFS = 12             # big x-tile pool slots (24 MB of SBUF)
F32 = mybir.dt.float32
AX = mybir.AxisListType.X
AF = mybir.ActivationFunctionType
MUL = mybir.AluOpType.mult

# wpack column layout (free dim), 128 partitions:
#   [0:256)    w1ts   lhsT chunks for y1 = w1 @ mean (mean divisor folded in)
#   [256:512)  w4t    w4.T
#   [512:640)  a2     A2
#   [640]      w2 broadcast   [641] w3 broadcast   [642] 1.0   [643] 0.0
#   [644:772)  row 0 holds 128 ones (lhsT for the partition-broadcast matmul)
WPACK_COLS = 772


def _build_nc():
    nc = bacc.Bacc(None, target_bir_lowering=False)

    x_ext = nc.declare_dram_parameter("x", [BPC, 2, P, HW], F32, isOutput=False)
    out_ext = nc.declare_dram_parameter("out", [BPC, 2, P, HW], F32, isOutput=True)
    wpack_ext = nc.declare_dram_parameter("wpack", [P, WPACK_COLS], F32,
                                          isOutput=False)

    with tile.TileContext(nc) as tc:
        with (
            tc.tile_pool(name="w", bufs=1) as wpool,
            tc.tile_pool(name="xp", bufs=XBUFS) as xpool,
            tc.tile_pool(name="sp", bufs=2) as spool,
            tc.tile_pool(name="pp", bufs=1, space=bass.MemorySpace.PSUM) as ppool,
        ):
            wpack = wpool.tile([P, WPACK_COLS], F32, tag="wpack")
            nc.gpsimd.dma_start(wpack[:], wpack_ext[:])

            # Warm-up ops consuming wpack on each compute engine: the engine
            # observes the wpack DMA semaphore here, so real instructions
            # below carry at most ONE sync wait each.
            warm = ppool.tile([1, 1], F32, tag="warm")
            nc.tensor.matmul(warm[:], wpack[0:1, 0:1], wpack[0:1, 0:1],
                             start=True, stop=True)
            wsc_a = spool.tile([P, 1], F32, tag="wsc_a")
            nc.scalar.activation(wsc_a[:], wpack[:, 643:644], AF.Exp,
                                 bias=wpack[:, 643:644], scale=1.0)
            wsc_s = spool.tile([P, 1], F32, tag="wsc_s")
            nc.scalar.activation(wsc_s[:], wpack[:, 643:644], AF.Sigmoid,
                                 bias=wpack[:, 643:644], scale=1.0)
            wsc_v = spool.tile([P, 1], F32, tag="wsc_v")
            nc.vector.tensor_copy(wsc_v[:], wpack[:, 643:644])

            w1ts = wpack[:, 0:C]
            w4t = wpack[:, C:2 * C]
            a2 = wpack[:, 2 * C:2 * C + P]
            w2v = wpack[:, 640:641]
            w3v = wpack[:, 641:642]
            ones = wpack[:, 642:643]
            zeros = wpack[:, 643:644]
            onesr = wpack[0:1, 644:772]

            def emit_load(b, u):
                h, ci = divmod(u, NCH)
                st, w = CHUNKS[ci]
                t = xpool.tile([P, w], F32, tag="x")
                nc.sync.dma_start(t[:], x_ext[b, h, :, st:st + w])
                return t

            def emit_reduce(acc, k, t):
                # one full-tile reduce; alternate engines so neither falls
                # behind the 4.65us/tile load pace
                if k % 2 == 0:
                    nc.vector.reduce_sum(acc[:, k:k + 1], t[:], axis=AX)
                else:
                    nc.scalar.activation(t[:], t[:], AF.Copy,
                                         accum_out=acc[:, k:k + 1])

            def emit_mul_store(b, u, t, gate, dve):
                h, ci = divmod(u, NCH)
                st, w = CHUNKS[ci]
                # first chunk's mul goes to DVE (2.8us vs 4.5us on ACT, and
                # DVE is idle right after the gate chain) so the store
                # stream starts sooner; later chunks keep the balanced
                # pairing with the other batch's reduces
                if dve or u == 0:
                    nc.vector.tensor_scalar_mul(t[:], t[:], gate[:, h:h + 1])
                else:
                    nc.scalar.mul(t[:], t[:], gate[:, h:h + 1])
                if b == 0:
                    steng = nc.scalar
                else:
                    steng = nc.sync if u % 2 == 0 else nc.scalar
                steng.dma_start(out_ext[b, h, :, st:st + w], t[:])

            def emit_gate(acc):
                # y1 = w1 @ mean: matmul straight on the per-chunk partial
                # sums (PSUM accumulates the channel halves), then one DVE
                # row-sum collapses the chunk axis PSUM->SBUF.
                y1p = ppool.tile([P, NCH], F32, tag="y1p")
                nc.tensor.matmul(y1p[:], w1ts[:, 0:HIDE], acc[:, 0:NCH],
                                 start=True, stop=False)
                nc.tensor.matmul(y1p[:], w1ts[:, HIDE:C],
                                 acc[:, NCH:2 * NCH],
                                 start=False, stop=True)
                y1 = spool.tile([P, 1], F32, tag="y1")
                nc.vector.reduce_sum(y1[:], y1p[:], axis=AX)

                # softmax(w2 * y1) over partitions (inputs are tiny -> no
                # max subtraction needed).  z = A2.T @ y1 and q = y1*e
                # overlap with the softmax-sum matmul chain.
                e = spool.tile([P, 1], F32, tag="e")
                nc.scalar.activation(e[:], y1[:], AF.Exp, bias=zeros, scale=w2v)
                zp = ppool.tile([P, 1], F32, tag="zp")
                nc.tensor.matmul(zp[:], a2[:], y1[:], start=True, stop=True)
                sump = ppool.tile([1, 1], F32, tag="sump")
                nc.tensor.matmul(sump[:], e[:], ones, start=True, stop=True)
                q = spool.tile([P, 1], F32, tag="q")
                nc.vector.tensor_mul(q[:], y1[:], e[:])
                r = spool.tile([1, 1], F32, tag="r")
                nc.vector.reciprocal(r[:], sump[:])
                rbp = ppool.tile([P, 1], F32, tag="rbp")
                nc.tensor.matmul(rbp[:], onesr[:], r[:], start=True, stop=True)

                # y2 = y1*softmax + A2.T@y1 = q/sum + z ; y3 = relu(w3*y2)
                y2 = spool.tile([P, 1], F32, tag="y2")
                nc.vector.tensor_mul(y2[:], q[:], rbp[:])
                nc.vector.tensor_add(y2[:], y2[:], zp[:])
                y3 = spool.tile([P, 1], F32, tag="y3")
                nc.vector.tensor_scalar(y3[:], y2[:], w3v, 0.0, MUL,
                                        mybir.AluOpType.max)

                # gate = sigmoid(w4 @ y3): two matmuls into one (128,2)
                # PSUM tile, one sigmoid over both columns.
                gp = ppool.tile([P, 2], F32, tag="gp")
                nc.tensor.matmul(gp[:, 0:1], w4t[:, 0:HIDE], y3[:],
                                 start=True, stop=True)
                nc.tensor.matmul(gp[:, 1:2], w4t[:, HIDE:C], y3[:],
                                 start=True, stop=True)
                gate = spool.tile([P, 2], F32, tag="gate")
                nc.scalar.activation(gate[:], gp[:], AF.Sigmoid,
                                     bias=zeros, scale=1.0)
                return gate

            NT = 2 * NCH
            acc0 = spool.tile([P, NT], F32, tag="acc0")
            tiles0 = []
            for k in range(NT):
                t = emit_load(0, k)
                emit_reduce(acc0, k, t)
                tiles0.append(t)

            gate0 = emit_gate(acc0)

            # Interleave batch 1 loads/reduces with batch 0 muls/stores so
            # each engine's instruction stream alternates between the two
            # (engine FIFOs execute in emission order).
            acc1 = spool.tile([P, NT], F32, tag="acc1")
            tiles1 = []
            for k in range(NT):
                t = emit_load(1, k)
                emit_reduce(acc1, k, t)
                tiles1.append(t)
                # mul on the engine the reduce did NOT use this step
                emit_mul_store(0, k, tiles0[k], gate0, dve=(k % 2 == 1))

            gate1 = emit_gate(acc1)
            for k in range(NT):
                emit_mul_store(1, k, tiles1[k], gate1, dve=(k % 2 == 1))

    nc.finalize()
    return nc


_NC_CACHE = {}


def _get_nc():
    if "nc" not in _NC_CACHE:
        _NC_CACHE["nc"] = _build_nc()
    return _NC_CACHE["nc"]


def _prep_in_maps(x, w1, w2, w3, w4, A2):
    x = np.ascontiguousarray(np.asarray(x, dtype=np.float32))
    w1 = np.asarray(w1, dtype=np.float32)
    w2 = float(np.asarray(w2))
    w3 = float(np.asarray(w3))
    w4 = np.asarray(w4, dtype=np.float32)
    A2 = np.asarray(A2, dtype=np.float32)

    wpack = np.zeros((P, WPACK_COLS), np.float32)
    # lhsT chunks for y1 = w1 @ (sums/HW): w1ts[k, h*HIDE+m] = w1[m, h*P+k]/HW
    w1t = (w1.T / float(HW)).astype(np.float32)          # (C, HIDE)
    wpack[:, 0:C] = w1t.reshape(2, P, HIDE).transpose(1, 0, 2).reshape(P, C)
    wpack[:, C:2 * C] = w4.T                             # (HIDE, C)
    wpack[:, 2 * C:2 * C + P] = A2
    wpack[:, 640] = w2
    wpack[:, 641] = w3
    wpack[:, 642] = 1.0
    wpack[:, 643] = 0.0
    wpack[0, 644:772] = 1.0

    in_maps = []
    for i in range(NCORES):
        shard = x[i * BPC:(i + 1) * BPC].reshape(BPC, 2, P, HW)
        in_maps.append({"x": shard, "wpack": wpack})
    return in_maps


def run(inputs, trace=False):
    """Run the kernel; returns (output, BassKernelResults)."""
    in_maps = _prep_in_maps(**inputs)
    nc = _get_nc()
    res = run_bass_kernel_spmd(nc, in_maps, core_ids=list(range(NCORES)),
                               trace=trace)
    out = np.empty((B, C, H, W), np.float32)
    for i in range(NCORES):
        out[i * BPC:(i + 1) * BPC] = np.asarray(
            res.results[i]["out"]).reshape(BPC, C, H, W)
    return out, res


def kernel(**inputs):
    out, _ = run(inputs, trace=False)
    return out
